# revision 27
# baseline (speedup 1.0000x reference)
"""Trainium2 kernel for nn_CIE_18236431138961 (embedding_lookup family).

Reference computation (per batch n, feature d):
    idx   = argsort-descending of x[n, :, d]            (S=16 sources)
    gaps  = consecutive differences of sorted values (last gap = last value)
    codes = cumulative bitmask of the top-k index set at each sort position
    table[c] = sum_j FM[source_index[c, j]] * Agg[0, j]  (c in [0, 2^S-1))
    out[n, :, d] = sum_s gaps[s] * table[codes[s]]       (a Choquet integral)

Key identity: the shipped source_index encodes row c as the bit pattern of
c+1, so table is ADDITIVE over bits:  table[c] = C + sum_{j in bits(c+1)} V[j]
with V[j] = table[{j}] - C and C = table[{0}]+table[{1}]-table[{0,1}].
For an additive (set-function) table the Choquet integral telescopes:
    sum_s gaps[s] * table[codes[s]]
      = sum_t x_sort[t] * V[idx[t]] + C * sum_s gaps[s]
      = sum_j x[n, j, d] * V[j]     + C * max_s x[n, s, d]
(the first term because idx is a permutation, the second because the gap sum
telescopes to the max).  With the reference FM (row 0 is the zero row) C == 0
exactly, and the whole pipeline is a single tiny contraction:
    out[n, h, d] = sum_s x[n, s, d] * V[s, h]

kernel() verifies this structure numerically on the host from the actual
inputs (so correctness never depends on the assumption), then runs the
contraction on 8 NeuronCores, data-parallel over the batch axis. If the
structure check ever fails (non-additive table), it falls back to a faithful
numpy implementation of the reference math.
"""

import numpy as np

N, S, D, H = 128, 16, 512, 4
NCORES = 8
NPC = N // NCORES          # batch rows per core
GROUPS = NPC // 8          # 8 batch rows per matmul (8*16 sources = 128 = K)

_BASS_CACHE = {}

# test.py hooks (harness never touches these)
TRACE = False
TRACE_KWARGS = {}
LAST_RESULTS = None


def _build_affine_nc():
    """Bass program (one NeuronCore, SPMD x8): out = blockdiag(V).T @ x.

    Inputs (per core):
      xs  [128, NPC*64] f32 : x shard rearranged so partition p = 16*j + s
                              (j = batch-in-group, s = source), free = (g, d)
      w   [128, 32] f32     : block-diagonal weights, w[16j+s, 4j+h] = V[s, h]
    Output:
      out [NPC*4, 512] f32  : rows g*32 + 4j + h  ->  out[8g+j, h, :]
    """
    import concourse.bass as bass
    import concourse.mybir as mybir
    from contextlib import ExitStack

    f32 = mybir.dt.float32
    # input layout: [ W (32 cols) | x_g0 (512) | x_g1 (512) ]  (W first so the
    # first DMA chunk carries it)
    NDUMMY = 8          # keep the PE HAM window busy until real data lands
    DWIDTH = 128        # narrow dummies so overshoot past data-arrival is small
    nc = bass.Bass()
    xw = nc.dram_tensor("xw", [128, 32 + GROUPS * 512], f32, kind="ExternalInput")
    out = nc.dram_tensor("out", [GROUPS * 32, 512], f32, kind="ExternalOutput")

    with ExitStack() as ctx:
        xt = ctx.enter_context(nc.sbuf_tensor([128, 32 + GROUPS * 512], f32))
        ot = ctx.enter_context(nc.sbuf_tensor([32, GROUPS * 512], f32))
        pts = [
            ctx.enter_context(nc.psum_tensor(f"pt{g}", [32, 512], f32))
            for g in range(GROUPS)
        ]
        pt1h = [
            ctx.enter_context(nc.psum_tensor(f"pt1h{h}", [32, 256], f32))
            for h in range(2)
        ]
        ptd = ctx.enter_context(nc.psum_tensor("ptd", [32, 256], f32))
        in_sems = [
            ctx.enter_context(nc.semaphore(f"in{g}")) for g in range(GROUPS)
        ]
        mm_sem = ctx.enter_context(nc.semaphore("mm"))
        cp_sem = ctx.enter_context(nc.semaphore("cp"))
        out_sem = ctx.enter_context(nc.semaphore("outs"))
        block = ctx.enter_context(nc.Block())

        @block.gpsimd
        def _(gpsimd):
            # chunk 0 = weights + group 0 via SWDGE (lower fixed latency, and
            # gpsimd clears the NEFF prologue earliest)
            gpsimd.dma_start(out=xt[:, 0:544], in_=xw[:, 0:544]).then_inc(
                in_sems[0], 16
            )

        @block.tensor
        def _(tensor):
            # keep the PE's HAM activity window alive until real data lands
            # (idling re-throttles the clock to 1.2 GHz)
            for i in range(NDUMMY):
                nc.tensor.matmul(
                    out=ptd[:, 0:DWIDTH],
                    lhsT=xt[:, 0:32],
                    rhs=xt[:, 544:544 + DWIDTH],
                    start=True,
                    stop=True,
                )
            tensor.wait_ge(in_sems[0], 16)
            nc.tensor.matmul(
                out=pts[0][:],
                lhsT=xt[:, 0:32],
                rhs=xt[:, 32:544],
                start=True,
                stop=True,
            ).then_inc(mm_sem, 1)
            # group 1 split in half (separate PSUM banks) so its first copy
            # overlaps its second matmul
            tensor.wait_ge(in_sems[1], 16)
            for h in range(2):
                nc.tensor.matmul(
                    out=pt1h[h][:],
                    lhsT=xt[:, 0:32],
                    rhs=xt[:, 544 + h * 256:544 + (h + 1) * 256],
                    start=True,
                    stop=True,
                ).then_inc(mm_sem, 1)

        @block.vector
        def _(vector):
            vector.wait_ge(mm_sem, 1)
            nc.vector.tensor_copy(
                out=ot[:, 0:512], in_=pts[0][:]
            ).then_inc(cp_sem, 1)
            for h in range(2):
                vector.wait_ge(mm_sem, 2 + h)
                nc.vector.tensor_copy(
                    out=ot[:, 512 + h * 256:512 + (h + 1) * 256],
                    in_=pt1h[h][:],
                ).then_inc(cp_sem, 1)

        @block.scalar
        def _(scalar):
            # remaining input chunks on the ACT HWDGE ring (parallel to SP's)
            for g in range(1, GROUPS):
                c0 = 32 + g * 512
                scalar.dma_start(
                    out=xt[:, c0:c0 + 512], in_=xw[:, c0:c0 + 512]
                ).then_inc(in_sems[g], 16)
            scalar.wait_ge(cp_sem, 1)
            scalar.dma_start(
                out=out[0:32, :], in_=ot[:, 0:512]
            ).then_inc(out_sem, 16)
            scalar.wait_ge(cp_sem, 3)
            scalar.dma_start(
                out=out[32:64, :], in_=ot[:, 512:1024]
            ).then_inc(out_sem, 16)
            scalar.wait_ge(out_sem, 32)

    # Strip the framework's init-time const-AP memsets and the all-engine
    # barrier that guards them (this kernel never reads the const APs; all
    # real dependencies are carried by our own semaphores). Engines then fall
    # straight through the entry block into the kernel, issuing the input
    # DMAs ~1us earlier.
    import concourse.mybir as mybir_m
    drop = (
        mybir_m.InstMemset,
        mybir_m.InstDrain,
        mybir_m.InstEventSemaphore,
    )
    blocks = nc.m.functions[0].blocks
    main_bb = blocks[0]
    assert main_bb.name == "main"
    main_bb.instructions = [
        i for i in main_bb.instructions if not isinstance(i, drop)
    ]
    for bb in blocks:
        if bb.name.endswith("_end"):
            bb.instructions = [
                i
                for i in bb.instructions
                if not isinstance(i, mybir_m.InstEventSemaphore)
            ]
    # Hoist each engine's leading input-DMA into `main`, just before that
    # engine's branch into its body block, so it issues without paying the
    # basic-block transition (~0.5us on gpsimd) and as early as possible.
    for bb in blocks:
        if bb.name == "main" or bb.name.endswith("_end"):
            continue
        if bb.instructions and isinstance(bb.instructions[0], mybir_m.InstDMACopy):
            dma = bb.instructions.pop(0)
            for pos, mi in enumerate(main_bb.instructions):
                if (
                    isinstance(mi, mybir_m.InstUnconditionalBranch)
                    and mi.engine == dma.engine
                ):
                    main_bb.instructions.insert(pos, dma)
                    break
    return nc


def _run_affine(x, V):
    """x (N,S,D) f32, V (S,H) f32 -> out (N,H,D) f32 via 8-core SPMD matmul."""
    global LAST_RESULTS
    from concourse.bass_utils import run_bass_kernel_spmd

    if "affine" not in _BASS_CACHE:
        _BASS_CACHE["affine"] = _build_affine_nc()
    nc = _BASS_CACHE["affine"]

    # block-diagonal lhsT: rows 16j+s, cols 4j+h
    w = np.zeros((128, 32), np.float32)
    for j in range(8):
        w[16 * j:16 * (j + 1), 4 * j:4 * (j + 1)] = V

    core_ids = list(range(NCORES))
    in_maps = []
    for c in core_ids:
        shard = x[c * NPC:(c + 1) * NPC]                  # (NPC, S, D)
        xs = shard.reshape(GROUPS, 128, 512).transpose(1, 0, 2).reshape(128, -1)
        in_maps.append({"xw": np.ascontiguousarray(np.concatenate([w, xs], axis=1))})

    res = run_bass_kernel_spmd(
        nc, in_maps, core_ids, trace=TRACE, **TRACE_KWARGS
    )
    LAST_RESULTS = res
    out = np.empty((N, H, D), np.float32)
    for c in core_ids:
        out[c * NPC:(c + 1) * NPC] = res.results[c]["out"].reshape(NPC, H, D)
    return out


def _general_fallback(x, table):
    """Faithful numpy mirror of the reference for non-additive tables."""
    idx = np.argsort(-x, axis=1, kind="stable")
    x_sort = np.take_along_axis(x, idx, axis=1)
    gaps = np.concatenate(
        [x_sort[:, :-1] - x_sort[:, 1:], x_sort[:, -1:]], axis=1
    )
    codes = np.cumsum((1 << idx.astype(np.int64)).astype(np.int32), axis=1) - 1
    fm = table[codes]                                     # (N,S,D,H)
    out = np.einsum("nsd,nsdh->ndh", gaps, fm)
    return np.ascontiguousarray(out.transpose(0, 2, 1).astype(np.float32))


def kernel(**inputs):
    x = np.ascontiguousarray(np.asarray(inputs["x"], dtype=np.float32))
    FM = np.asarray(inputs["FM"], dtype=np.float32)
    Agg = np.asarray(inputs["Agg"], dtype=np.float32)
    si = np.asarray(inputs["source_index"])

    # Host-side param preprocessing: per-code reduction table (65535, H).
    table = (FM[si] * Agg[0][None, :, :]).sum(1).astype(np.float32)

    # Affine fit over the bit pattern of c+1.
    C = table[0] + table[1] - table[2]                    # {0}+{1}-{0,1}
    V = table[(1 << np.arange(S)) - 1] - C                # (S, H) singletons
    bits = ((np.arange(1, 2 ** S)[:, None] >> np.arange(S)) & 1).astype(
        np.float32
    )
    recon = C[None, :] + bits @ V
    scale = max(float(np.abs(table).max()), 1e-12)
    affine = float(np.abs(recon - table).max()) <= 1e-4 * scale
    c_zero = float(np.abs(C).max()) <= 1e-5 * scale

    if affine and c_zero:
        return _run_affine(x, V.astype(np.float32))
    return _general_fallback(x, table)
